# revision 1
# baseline (speedup 1.0000x reference)
"""nn_MoE_57492432224434 — MoE (SwiGLU, top-2 of 8 experts) on 8 TRN2 NeuronCores.

Strategy (expert-parallel, matching the sharding hint):
  * Host computes the tiny router (logits = x @ gw.T, top-2, softmax) and
    dispatches tokens: core e receives the tokens routed to expert e
    (transposed, zero-padded to capacity C), plus expert e's weights
    pre-transposed/pre-tiled so every device load is one contiguous DMA.
  * Each core runs a Bass/Tile kernel computing, entirely in float32r
    (full-speed PE mode, ~1e-4 rel err):
        h1T = (x @ w1.T).T ; h2T = (x @ w2.T).T        [PE]
        gT  = silu(h1T) * h2T                          [ACT + DVE]
        yT  = (g @ w3.T).T scaled by combine weight    [PE + DVE]
  * Host scatter-adds each core's yT columns back into the [T, D] output.

All layouts are transposed (tokens on the free axis) so no on-device
transposes are needed anywhere.
"""
import numpy as np
import orjson

import concourse.bass as bass
import concourse.mybir as mybir
import concourse.tile as tile

# ---------------------------------------------------------------------------
# Workaround for this container's walrus build: any instruction carrying more
# than ONE sync-wait command is rejected ("Too many sync wait commands").
# Tile's semaphore assignment routinely attaches several waits to one
# instruction; split the extras onto preceding NOPs on the same engine (same
# basic block, so per-engine program order is preserved).
# ---------------------------------------------------------------------------

def _legalize_bir_json(bir_json: bytes) -> bytes:
    bir = orjson.loads(bir_json)
    for fn in bir.get("functions", []):
        for bb in fn.get("blocks", []):
            out = []
            for inst in bb.get("instructions", []):
                si = inst.get("sync_info")
                waits = si.get("on_wait") if si else None
                if waits and len(waits) > 1:
                    for i, w in enumerate(waits[:-1]):
                        nop = {
                            "engine": inst["engine"], "ins": [], "outs": [],
                            "name": f"{inst['name']}_lw{i}", "opcode": "NoOp",
                            "sync_info": {"on_update": [], "on_wait": [w]},
                        }
                        if "debug" in inst:
                            nop["debug"] = inst["debug"]
                        out.append(nop)
                    si["on_wait"] = [waits[-1]]
                out.append(inst)
            bb["instructions"] = out
    return orjson.dumps(bir)


def _install_legalizer():
    import concourse.bass_utils as bu
    import concourse.bass2jax as b2j
    if getattr(bu.compile_bir_kernel, "_legalized", False):
        return
    orig = bu.compile_bir_kernel

    def wrapped(bir_json, tmpdir, neff_name="file.neff"):
        return orig(_legalize_bir_json(bytes(bir_json)), tmpdir, neff_name=neff_name)

    wrapped._legalized = True
    bu.compile_bir_kernel = wrapped
    b2j.compile_bir_kernel = wrapped


_install_legalizer()

# ---------------------------------------------------------------------------
# Jit-once SPMD runner over axon PJRT (run_bass_kernel_spmd re-jits per call).
# ---------------------------------------------------------------------------

class SpmdRunner:
    def __init__(self, nc, n_cores):
        import jax
        from jax.experimental.shard_map import shard_map
        from jax.sharding import Mesh, PartitionSpec
        import concourse.bass2jax as b2j
        b2j.install_neuronx_cc_hook()
        self.n_cores = n_cores
        partition_name = nc.partition_id_tensor.name if nc.partition_id_tensor else None
        in_names, out_names, out_avals = [], [], []
        for alloc in nc.m.functions[0].allocations:
            if not isinstance(alloc, mybir.MemoryLocationSet):
                continue
            name = alloc.memorylocations[0].name
            if alloc.kind == "ExternalInput":
                if name != partition_name:
                    in_names.append(name)
            elif alloc.kind == "ExternalOutput":
                out_names.append(name)
                out_avals.append(jax.core.ShapedArray(tuple(alloc.tensor_shape),
                                                      mybir.dt.np(alloc.dtype)))
        self.in_names, self.out_names, self.out_avals = in_names, out_names, out_avals
        n_params = len(in_names)

        def _body(*args):
            operands = list(args)
            if partition_name is not None:
                operands.append(b2j.partition_id_tensor())
            outs = b2j._bass_exec_p.bind(
                *operands,
                out_avals=tuple(out_avals),
                in_names=tuple(list(in_names) + list(out_names) +
                               ([partition_name] if partition_name else [])),
                out_names=tuple(out_names),
                lowering_input_output_aliases=(),
                sim_require_finite=False, sim_require_nnan=False, nc=nc,
            )
            return tuple(outs)

        devices = jax.devices()[:n_cores]
        assert len(devices) == n_cores, f"need {n_cores} cores, have {len(devices)}"
        mesh = Mesh(np.asarray(devices), ("core",))
        nz = len(out_names)
        self._fn = jax.jit(
            shard_map(_body, mesh=mesh,
                      in_specs=(PartitionSpec("core"),) * (n_params + nz),
                      out_specs=(PartitionSpec("core"),) * nz,
                      check_rep=False),
            keep_unused=True,
        )
        self._zeros = [
            jax.device_put(np.zeros((n_cores * a.shape[0], *a.shape[1:]), a.dtype))
            for a in out_avals
        ]
        self._jax = jax

    def run(self, in_maps):
        jax = self._jax
        concat = [
            np.concatenate([np.asarray(in_maps[c][n]) for c in range(self.n_cores)], axis=0)
            for n in self.in_names
        ]
        dev = [jax.device_put(a) for a in concat]
        outs = [np.asarray(o) for o in self._fn(*dev, *self._zeros)]
        return [
            {n: outs[i].reshape(self.n_cores, *self.out_avals[i].shape)[c]
             for i, n in enumerate(self.out_names)}
            for c in range(self.n_cores)
        ]


# ---------------------------------------------------------------------------
# Problem constants (hardcoded per the harness contract) and kernel builder.
# ---------------------------------------------------------------------------

D = 1024          # model dim
F = 2816          # expert hidden dim
E = 8             # experts == cores
TOPK = 2
C_DEFAULT = 1280  # per-expert token capacity (multiple of 128; observed max ~1078)
DT = D // 128
FT = F // 128
FP32R = mybir.dt.float32r
FP32 = mybir.dt.float32


def _tok_tiles(C):
    """Split C into tiles of <=512 columns, all >=256 when possible —
    float32r matmuls drop to 1/4 rate below a 256-wide moving operand."""
    tiles, t0 = [], 0
    while C - t0 > 512:
        n = 512 if (C - t0) % 512 != 128 else 384
        tiles.append((t0, n))
        t0 += n
    tiles.append((t0, C - t0))
    return tiles


def build(C):
    TOK = _tok_tiles(C)
    YPW = -(-C // 512) * 512  # yp psum tile width, bank-aligned
    nc = bass.Bass(target_bir_lowering=False)
    xt = nc.dram_tensor("xt", [D, C], FP32R, kind="ExternalInput")
    w1p = nc.dram_tensor("w1p", [FT, D, 128], FP32R, kind="ExternalInput")
    w2p = nc.dram_tensor("w2p", [FT, D, 128], FP32R, kind="ExternalInput")
    w3p = nc.dram_tensor("w3p", [DT, F, 128], FP32R, kind="ExternalInput")
    cw = nc.dram_tensor("cw", [128, C], FP32, kind="ExternalInput")
    yt = nc.dram_tensor("yt", [D, C], FP32, kind="ExternalOutput")

    with tile.TileContext(nc) as tc:
        with (
            tc.tile_pool(name="resident", bufs=1) as rpool,
            tc.tile_pool(name="stream", bufs=2) as spool,
            tc.tile_pool(name="work", bufs=2) as wpool,
            tc.tile_pool(name="psum", bufs=1, space="PSUM") as ppool,
        ):
            xsb = rpool.tile([128, DT * C], FP32R)   # x.T: d-chunk k at cols [k*C, (k+1)*C)
            gsb = rpool.tile([128, FT * C], FP32R)   # g.T: f-chunk f at cols [f*C, (f+1)*C)
            cwsb = rpool.tile([128, C], FP32)
            nc.sync.dma_start(out=cwsb[:, :], in_=cw[:, :])
            for k in range(DT):
                nc.sync.dma_start(out=xsb[:, bass.ds(k * C, C)], in_=xt[k*128:(k+1)*128, :])

            # phase A: gT = silu((x @ w1.T).T) * (x @ w2.T).T
            for f in range(FT):
                w1sb = spool.tile([128, DT * 128], FP32R, tag="w1sb")
                w2sb = spool.tile([128, DT * 128], FP32R, tag="w2sb")
                nc.sync.dma_start(out=w1sb.rearrange("p (k m) -> p k m", k=DT),
                                  in_=w1p[f].rearrange("(k p) m -> p k m", p=128))
                nc.sync.dma_start(out=w2sb.rearrange("p (k m) -> p k m", k=DT),
                                  in_=w2p[f].rearrange("(k p) m -> p k m", p=128))
                for (t0, tn) in TOK:
                    h1 = ppool.tile([128, 512], FP32, tag="h1", bufs=1)
                    h2 = ppool.tile([128, 512], FP32, tag="h2", bufs=1)
                    for k in range(DT):
                        nc.tensor.matmul(h1[:, :tn], w1sb[:, bass.ts(k, 128)],
                                         xsb[:, bass.ds(k * C + t0, tn)],
                                         start=(k == 0), stop=(k == DT - 1))
                    for k in range(DT):
                        nc.tensor.matmul(h2[:, :tn], w2sb[:, bass.ts(k, 128)],
                                         xsb[:, bass.ds(k * C + t0, tn)],
                                         start=(k == 0), stop=(k == DT - 1))
                    smu = wpool.tile([128, 512], FP32, tag="smu")
                    nc.scalar.activation(smu[:, :tn], h1[:, :tn],
                                         mybir.ActivationFunctionType.Silu)
                    nc.vector.tensor_mul(gsb[:, bass.ds(f * C + t0, tn)],
                                         smu[:, :tn], h2[:, :tn])

            # phase B: yT[d,:] = sum_f w3t-block.T @ gT, scaled by combine weight
            for d in range(DT):
                w3sb = spool.tile([128, FT * 128], FP32R, tag="w3sb")
                nc.sync.dma_start(out=w3sb.rearrange("p (f m) -> p f m", f=FT),
                                  in_=w3p[d].rearrange("(f p) m -> p f m", p=128))
                yp = ppool.tile([128, YPW], FP32, tag="yp", bufs=2)
                for f in range(FT):
                    for (t0, tn) in TOK:
                        nc.tensor.matmul(yp[:, bass.ds(t0, tn)], w3sb[:, bass.ts(f, 128)],
                                         gsb[:, bass.ds(f * C + t0, tn)],
                                         start=(f == 0), stop=(f == FT - 1))
                osb = wpool.tile([128, C], FP32, tag="osb", bufs=2)
                nc.vector.tensor_mul(osb[:, :], yp[:, :C], cwsb[:, :])
                nc.sync.dma_start(out=yt[d*128:(d+1)*128, :], in_=osb[:, :])
    return nc


# ---------------------------------------------------------------------------
# Host routing / dispatch / combine
# ---------------------------------------------------------------------------

def _route(x, gw):
    logits = x @ gw.T                                    # [T, E]
    order = np.argsort(-logits, axis=1, kind="stable")   # ties -> lower idx, as top_k
    idx = order[:, :TOPK]
    vals = np.take_along_axis(logits, idx, axis=1)
    ex = np.exp(vals - vals[:, :1])
    sv = ex / ex.sum(axis=1, keepdims=True)
    per_expert = []
    for e in range(E):
        mask = idx == e
        tok = np.nonzero(mask.any(axis=1))[0]
        per_expert.append((tok, sv[mask]))
    return per_expert


_runners = {}


def _get_runner(C):
    if C not in _runners:
        _runners[C] = SpmdRunner(build(C), E)
    return _runners[C]


def kernel(xmat, gw, w1, w2, w3):
    B, L, d = xmat.shape
    x = np.ascontiguousarray(np.asarray(xmat, dtype=np.float32).reshape(-1, d))
    gw = np.asarray(gw, dtype=np.float32)
    w1 = np.asarray(w1, dtype=np.float32)
    w2 = np.asarray(w2, dtype=np.float32)
    w3 = np.asarray(w3, dtype=np.float32)

    per_expert = _route(x, gw)
    max_n = max(len(tok) for tok, _ in per_expert)
    C = max(C_DEFAULT, -(-max_n // 128) * 128)
    in_maps = []
    for e in range(E):
        tok, w = per_expert[e]
        n = len(tok)
        xt = np.zeros((D, C), np.float32)
        xt[:, :n] = x[tok].T
        cwrow = np.zeros((1, C), np.float32)
        cwrow[0, :n] = w
        in_maps.append({
            "xt": xt,
            "w1p": np.ascontiguousarray(w1[e].T.reshape(D, FT, 128).transpose(1, 0, 2)),
            "w2p": np.ascontiguousarray(w2[e].T.reshape(D, FT, 128).transpose(1, 0, 2)),
            "w3p": np.ascontiguousarray(w3[e].T.reshape(F, DT, 128).transpose(1, 0, 2)),
            "cw": np.ascontiguousarray(np.broadcast_to(cwrow, (128, C))),
        })

    results = _get_runner(C).run(in_maps)

    y = np.zeros((x.shape[0], D), np.float32)
    for e in range(E):
        tok, _ = per_expert[e]
        y[tok] += results[e]["yt"][:, :len(tok)].T
    return y.reshape(B, L, d)



# revision 2
# speedup vs baseline: 6.5106x; 6.5106x over previous
"""nn_MoE_57492432224434 — MoE (SwiGLU, top-2 of 8 experts) on 8 TRN2 NeuronCores.

Strategy (expert-parallel, matching the sharding hint):
  * Host computes the tiny router (logits = x @ gw.T, top-2, softmax) and
    dispatches tokens: core e receives the tokens routed to expert e
    (transposed, zero-padded to capacity C = max expert load rounded up to
    even), plus expert e's weights pre-transposed/packed so every device
    load is one fully contiguous per-partition DMA.
  * All matmul operands are bf16 (same PE rate as fp32r on trn2 — 1 col per
    cycle at >=256-wide moving operands — but half the DMA/SBUF traffic);
    accumulation happens in fp32 PSUM.
  * Each core runs a Bass/Tile kernel:
        h1T/h2T = (x @ w1.T).T, (x @ w2.T).T   [PE, fused w12 weight stream]
        gT  = silu(h1T) * h2T                  [ACT + DVE]
        yT  = (g @ w3.T).T scaled by combine   [PE + DVE], bf16 out
  * Host scatter-adds each core's yT columns back into the [T, D] output.

All layouts are transposed (tokens on the free axis) so no on-device
transposes are needed anywhere.
"""
import numpy as np
import orjson

import concourse.bass as bass
import concourse.mybir as mybir
import concourse.tile as tile

# ---------------------------------------------------------------------------
# Workaround for this container's walrus build: any instruction carrying more
# than ONE sync-wait command is rejected ("Too many sync wait commands").
# Tile's semaphore assignment routinely attaches several waits to one
# instruction; split the extras onto preceding NOPs on the same engine (same
# basic block, so per-engine program order is preserved).
# ---------------------------------------------------------------------------

def _legalize_bir_json(bir_json: bytes) -> bytes:
    bir = orjson.loads(bir_json)
    for fn in bir.get("functions", []):
        for bb in fn.get("blocks", []):
            out = []
            for inst in bb.get("instructions", []):
                si = inst.get("sync_info")
                waits = si.get("on_wait") if si else None
                if waits and len(waits) > 1:
                    for i, w in enumerate(waits[:-1]):
                        nop = {
                            "engine": inst["engine"], "ins": [], "outs": [],
                            "name": f"{inst['name']}_lw{i}", "opcode": "NoOp",
                            "sync_info": {"on_update": [], "on_wait": [w]},
                        }
                        if "debug" in inst:
                            nop["debug"] = inst["debug"]
                        out.append(nop)
                    si["on_wait"] = [waits[-1]]
                out.append(inst)
            bb["instructions"] = out
    return orjson.dumps(bir)


def _install_legalizer():
    import concourse.bass_utils as bu
    import concourse.bass2jax as b2j
    if getattr(bu.compile_bir_kernel, "_legalized", False):
        return
    orig = bu.compile_bir_kernel

    def wrapped(bir_json, tmpdir, neff_name="file.neff"):
        return orig(_legalize_bir_json(bytes(bir_json)), tmpdir, neff_name=neff_name)

    wrapped._legalized = True
    bu.compile_bir_kernel = wrapped
    b2j.compile_bir_kernel = wrapped


_install_legalizer()

# ---------------------------------------------------------------------------
# Jit-once SPMD runner over axon PJRT (run_bass_kernel_spmd re-jits per call).
# ---------------------------------------------------------------------------

class SpmdRunner:
    def __init__(self, nc, n_cores):
        import jax
        from jax.experimental.shard_map import shard_map
        from jax.sharding import Mesh, PartitionSpec
        import concourse.bass2jax as b2j
        b2j.install_neuronx_cc_hook()
        self.n_cores = n_cores
        partition_name = nc.partition_id_tensor.name if nc.partition_id_tensor else None
        in_names, out_names, out_avals = [], [], []
        for alloc in nc.m.functions[0].allocations:
            if not isinstance(alloc, mybir.MemoryLocationSet):
                continue
            name = alloc.memorylocations[0].name
            if alloc.kind == "ExternalInput":
                if name != partition_name:
                    in_names.append(name)
            elif alloc.kind == "ExternalOutput":
                out_names.append(name)
                out_avals.append(jax.core.ShapedArray(tuple(alloc.tensor_shape),
                                                      mybir.dt.np(alloc.dtype)))
        self.in_names, self.out_names, self.out_avals = in_names, out_names, out_avals
        n_params = len(in_names)

        def _body(*args):
            operands = list(args)
            if partition_name is not None:
                operands.append(b2j.partition_id_tensor())
            outs = b2j._bass_exec_p.bind(
                *operands,
                out_avals=tuple(out_avals),
                in_names=tuple(list(in_names) + list(out_names) +
                               ([partition_name] if partition_name else [])),
                out_names=tuple(out_names),
                lowering_input_output_aliases=(),
                sim_require_finite=False, sim_require_nnan=False, nc=nc,
            )
            return tuple(outs)

        devices = jax.devices()[:n_cores]
        assert len(devices) == n_cores, f"need {n_cores} cores, have {len(devices)}"
        mesh = Mesh(np.asarray(devices), ("core",))
        nz = len(out_names)
        self._fn = jax.jit(
            shard_map(_body, mesh=mesh,
                      in_specs=(PartitionSpec("core"),) * (n_params + nz),
                      out_specs=(PartitionSpec("core"),) * nz,
                      check_rep=False),
            keep_unused=True,
        )
        self._zeros = [
            jax.device_put(np.zeros((n_cores * a.shape[0], *a.shape[1:]), a.dtype))
            for a in out_avals
        ]
        self._jax = jax

    def put_inputs(self, in_maps):
        jax = self._jax
        concat = [
            np.concatenate([np.asarray(in_maps[c][n]) for c in range(self.n_cores)], axis=0)
            for n in self.in_names
        ]
        return [jax.device_put(a) for a in concat]

    def execute(self, dev):
        return self._fn(*dev, *self._zeros)

    def run(self, in_maps):
        dev = self.put_inputs(in_maps)
        outs = [np.asarray(o) for o in self.execute(dev)]
        return [
            {n: outs[i].reshape(self.n_cores, *self.out_avals[i].shape)[c]
             for i, n in enumerate(self.out_names)}
            for c in range(self.n_cores)
        ]


# ---------------------------------------------------------------------------
# Problem constants (hardcoded per the harness contract) and kernel builder.
# ---------------------------------------------------------------------------

D = 1024          # model dim
F = 2816          # expert hidden dim
E = 8             # experts == cores
TOPK = 2
DT = D // 128
FT = F // 128
BF16 = mybir.dt.bfloat16
FP32 = mybir.dt.float32
NPBF16 = mybir.dt.np(BF16)


def _tok_tiles(C):
    """Tiles of <=512 columns (PSUM bank width), all >=256 when possible
    (full-rate moving operands) and all even with even start offsets."""
    assert C % 2 == 0, C
    tiles, t0 = [], 0
    rem = C
    while rem > 768:
        tiles.append((t0, 512))
        t0 += 512
        rem -= 512
    if rem > 512:
        n1 = (rem - 256) & ~1
        tiles.append((t0, n1))
        tiles.append((t0 + n1, rem - n1))
    else:
        tiles.append((t0, rem))
    return tiles


def build(C=1072, n_copies=1):
    TOK = _tok_tiles(C)
    nc = bass.Bass(target_bir_lowering=False)
    xt = nc.dram_tensor("xt", [D, C], BF16, kind="ExternalInput")
    w12p = nc.dram_tensor("w12p", [FT, 128, 2 * DT * 128], BF16, kind="ExternalInput")
    w3p = nc.dram_tensor("w3p", [DT, 128, FT * 128], BF16, kind="ExternalInput")
    cw = nc.dram_tensor("cw", [128, C], FP32, kind="ExternalInput")
    yts = [nc.dram_tensor("yt" if i == 0 else f"yt{i}", [D, C], BF16,
                          kind="ExternalOutput") for i in range(n_copies)]

    with tile.TileContext(nc) as tc:
        with (
            tc.tile_pool(name="resident", bufs=1) as rpool,
            tc.tile_pool(name="stream", bufs=3) as spool,
            tc.tile_pool(name="work", bufs=2) as wpool,
            tc.tile_pool(name="psum", bufs=1, space="PSUM") as ppool,
        ):
            for yt in yts:
                xsb = rpool.tile([128, DT * C], BF16, tag="xsb")
                gsb = rpool.tile([128, FT * C], BF16, tag="gsb")
                cwsb = rpool.tile([128, C], FP32, tag="cwsb")

                # phase A: gT = silu((x @ w1.T).T) * (x @ w2.T).T
                for f in range(FT):
                    w12sb = spool.tile([128, 2 * DT * 128], BF16, tag="w12sb")
                    nc.sync.dma_start(out=w12sb[:, :], in_=w12p[f])
                    if f == 0:
                        for k in range(DT):
                            nc.sync.dma_start(out=xsb[:, bass.ds(k * C, C)],
                                              in_=xt[k*128:(k+1)*128, :])
                        nc.sync.dma_start(out=cwsb[:, :], in_=cw[:, :])
                    for (t0, tn) in TOK:
                        h1 = ppool.tile([128, 512], FP32, tag="h1", bufs=1)
                        h2 = ppool.tile([128, 512], FP32, tag="h2", bufs=1)
                        for k in range(DT):
                            nc.tensor.matmul(h1[:, :tn], w12sb[:, bass.ts(k, 128)],
                                             xsb[:, bass.ds(k * C + t0, tn)],
                                             start=(k == 0), stop=(k == DT - 1))
                        for k in range(DT):
                            nc.tensor.matmul(h2[:, :tn],
                                             w12sb[:, bass.ds(DT * 128 + k * 128, 128)],
                                             xsb[:, bass.ds(k * C + t0, tn)],
                                             start=(k == 0), stop=(k == DT - 1))
                        smu = wpool.tile([128, 512], FP32, tag="smu")
                        nc.scalar.activation(smu[:, :tn], h1[:, :tn],
                                             mybir.ActivationFunctionType.Silu)
                        nc.vector.tensor_mul(gsb[:, bass.ds(f * C + t0, tn)],
                                             smu[:, :tn], h2[:, :tn])

                # phase B: yT[d-block] = sum_f w3-block @ g-block, * combine
                for d in range(DT):
                    w3sb = spool.tile([128, FT * 128], BF16, tag="w3sb")
                    nc.sync.dma_start(out=w3sb[:, :], in_=w3p[d])
                    osb = wpool.tile([128, C], BF16, tag="osb", bufs=2)
                    for ti, (t0, tn) in enumerate(TOK):
                        yp = ppool.tile([128, 512], FP32, tag=f"yp{ti}", bufs=2)
                        for f in range(FT):
                            nc.tensor.matmul(yp[:, :tn], w3sb[:, bass.ts(f, 128)],
                                             gsb[:, bass.ds(f * C + t0, tn)],
                                             start=(f == 0), stop=(f == FT - 1))
                        nc.vector.tensor_mul(osb[:, bass.ds(t0, tn)],
                                             yp[:, :tn], cwsb[:, bass.ds(t0, tn)])
                        nc.sync.dma_start(out=yt[d*128:(d+1)*128, t0:t0 + tn],
                                          in_=osb[:, bass.ds(t0, tn)])
    return nc


# ---------------------------------------------------------------------------
# Host routing / dispatch / combine
# ---------------------------------------------------------------------------

def _route(x, gw):
    logits = x @ gw.T                                    # [T, E]
    order = np.argsort(-logits, axis=1, kind="stable")   # ties -> lower idx, as top_k
    idx = order[:, :TOPK]
    vals = np.take_along_axis(logits, idx, axis=1)
    ex = np.exp(vals - vals[:, :1])
    sv = ex / ex.sum(axis=1, keepdims=True)
    per_expert = []
    for e in range(E):
        mask = idx == e
        tok = np.nonzero(mask.any(axis=1))[0]
        per_expert.append((tok, sv[mask]))
    return per_expert


_runners = {}


def _get_runner(C):
    if C not in _runners:
        _runners[C] = SpmdRunner(build(C), E)
    return _runners[C]


def kernel(xmat, gw, w1, w2, w3):
    B, L, d = xmat.shape
    x = np.ascontiguousarray(np.asarray(xmat, dtype=np.float32).reshape(-1, d))
    gw = np.asarray(gw, dtype=np.float32)

    per_expert = _route(x, gw)
    max_n = max(len(tok) for tok, _ in per_expert)
    C = max(256, max_n + (max_n & 1))
    in_maps = []
    for e in range(E):
        tok, w = per_expert[e]
        n = len(tok)
        xtp = np.zeros((D, C), NPBF16)
        xtp[:, :n] = x[tok].T.astype(NPBF16)
        cwrow = np.zeros((1, C), np.float32)
        cwrow[0, :n] = w
        a1 = np.asarray(w1[e], np.float32).reshape(FT, 128, DT, 128).transpose(0, 3, 2, 1)
        a2 = np.asarray(w2[e], np.float32).reshape(FT, 128, DT, 128).transpose(0, 3, 2, 1)
        w12 = np.concatenate([a1.reshape(FT, 128, DT * 128),
                              a2.reshape(FT, 128, DT * 128)], axis=2).astype(NPBF16)
        b3 = np.asarray(w3[e], np.float32).reshape(DT, 128, FT, 128).transpose(0, 3, 2, 1)
        in_maps.append({
            "xt": xtp,
            "w12p": w12,
            "w3p": np.ascontiguousarray(b3.reshape(DT, 128, FT * 128)).astype(NPBF16),
            "cw": np.ascontiguousarray(np.broadcast_to(cwrow, (128, C))),
        })

    results = _get_runner(C).run(in_maps)

    y = np.zeros((x.shape[0], D), np.float32)
    for e in range(E):
        tok, _ = per_expert[e]
        y[tok] += results[e]["yt"][:, :len(tok)].T.astype(np.float32)
    return y.reshape(B, L, d)


# revision 3
# speedup vs baseline: 7.6552x; 1.1758x over previous
"""nn_MoE_57492432224434 — MoE (SwiGLU, top-2 of 8 experts) on 8 TRN2 NeuronCores.

Strategy (expert-parallel, matching the sharding hint):
  * Host computes the tiny router (logits = x @ gw.T, top-2, softmax) and
    dispatches tokens: core e receives the tokens routed to expert e
    (transposed, zero-padded to capacity C = max expert load rounded up to
    even), plus expert e's weights pre-transposed/packed so every device
    load is one fully contiguous per-partition DMA.
  * All matmul operands are bf16 (same PE rate as fp32r on trn2 — 1 col per
    cycle at >=256-wide moving operands — but half the DMA/SBUF traffic);
    accumulation happens in fp32 PSUM.
  * Each core runs a Bass/Tile kernel:
        h1T/h2T = (x @ w1.T).T, (x @ w2.T).T   [PE, fused w12 weight stream]
        gT  = silu(h1T) * h2T                  [ACT + DVE]
        yT  = (g @ w3.T).T scaled by combine   [PE + DVE], bf16 out
  * Host scatter-adds each core's yT columns back into the [T, D] output.

All layouts are transposed (tokens on the free axis) so no on-device
transposes are needed anywhere.
"""
import numpy as np
import orjson

import concourse.bass as bass
import concourse.mybir as mybir
import concourse.tile as tile

# ---------------------------------------------------------------------------
# Workaround for this container's walrus build: any instruction carrying more
# than ONE sync-wait command is rejected ("Too many sync wait commands").
# Tile's semaphore assignment routinely attaches several waits to one
# instruction; split the extras onto preceding NOPs on the same engine (same
# basic block, so per-engine program order is preserved).
# ---------------------------------------------------------------------------

def _legalize_bir_json(bir_json: bytes) -> bytes:
    bir = orjson.loads(bir_json)
    for fn in bir.get("functions", []):
        for bb in fn.get("blocks", []):
            out = []
            for inst in bb.get("instructions", []):
                si = inst.get("sync_info")
                waits = si.get("on_wait") if si else None
                if waits and len(waits) > 1:
                    for i, w in enumerate(waits[:-1]):
                        nop = {
                            "engine": inst["engine"], "ins": [], "outs": [],
                            "name": f"{inst['name']}_lw{i}", "opcode": "NoOp",
                            "sync_info": {"on_update": [], "on_wait": [w]},
                        }
                        if "debug" in inst:
                            nop["debug"] = inst["debug"]
                        out.append(nop)
                    si["on_wait"] = [waits[-1]]
                out.append(inst)
            bb["instructions"] = out
    return orjson.dumps(bir)


def _install_legalizer():
    import concourse.bass_utils as bu
    import concourse.bass2jax as b2j
    if getattr(bu.compile_bir_kernel, "_legalized", False):
        return
    orig = bu.compile_bir_kernel

    def wrapped(bir_json, tmpdir, neff_name="file.neff"):
        return orig(_legalize_bir_json(bytes(bir_json)), tmpdir, neff_name=neff_name)

    wrapped._legalized = True
    bu.compile_bir_kernel = wrapped
    b2j.compile_bir_kernel = wrapped


_install_legalizer()

# ---------------------------------------------------------------------------
# Jit-once SPMD runner over axon PJRT (run_bass_kernel_spmd re-jits per call).
# ---------------------------------------------------------------------------

class SpmdRunner:
    def __init__(self, nc, n_cores):
        import jax
        from jax.experimental.shard_map import shard_map
        from jax.sharding import Mesh, PartitionSpec
        import concourse.bass2jax as b2j
        b2j.install_neuronx_cc_hook()
        self.n_cores = n_cores
        partition_name = nc.partition_id_tensor.name if nc.partition_id_tensor else None
        in_names, out_names, out_avals = [], [], []
        for alloc in nc.m.functions[0].allocations:
            if not isinstance(alloc, mybir.MemoryLocationSet):
                continue
            name = alloc.memorylocations[0].name
            if alloc.kind == "ExternalInput":
                if name != partition_name:
                    in_names.append(name)
            elif alloc.kind == "ExternalOutput":
                out_names.append(name)
                out_avals.append(jax.core.ShapedArray(tuple(alloc.tensor_shape),
                                                      mybir.dt.np(alloc.dtype)))
        self.in_names, self.out_names, self.out_avals = in_names, out_names, out_avals
        n_params = len(in_names)

        def _body(*args):
            operands = list(args)
            if partition_name is not None:
                operands.append(b2j.partition_id_tensor())
            outs = b2j._bass_exec_p.bind(
                *operands,
                out_avals=tuple(out_avals),
                in_names=tuple(list(in_names) + list(out_names) +
                               ([partition_name] if partition_name else [])),
                out_names=tuple(out_names),
                lowering_input_output_aliases=(),
                sim_require_finite=False, sim_require_nnan=False, nc=nc,
            )
            return tuple(outs)

        devices = jax.devices()[:n_cores]
        assert len(devices) == n_cores, f"need {n_cores} cores, have {len(devices)}"
        mesh = Mesh(np.asarray(devices), ("core",))
        nz = len(out_names)
        self._fn = jax.jit(
            shard_map(_body, mesh=mesh,
                      in_specs=(PartitionSpec("core"),) * (n_params + nz),
                      out_specs=(PartitionSpec("core"),) * nz,
                      check_rep=False),
            keep_unused=True,
        )
        self._zeros = [
            jax.device_put(np.zeros((n_cores * a.shape[0], *a.shape[1:]), a.dtype))
            for a in out_avals
        ]
        self._jax = jax

    def put_inputs(self, in_maps):
        jax = self._jax
        concat = [
            np.concatenate([np.asarray(in_maps[c][n]) for c in range(self.n_cores)], axis=0)
            for n in self.in_names
        ]
        return [jax.device_put(a) for a in concat]

    def execute(self, dev):
        return self._fn(*dev, *self._zeros)

    def run(self, in_maps):
        dev = self.put_inputs(in_maps)
        outs = [np.asarray(o) for o in self.execute(dev)]
        return [
            {n: outs[i].reshape(self.n_cores, *self.out_avals[i].shape)[c]
             for i, n in enumerate(self.out_names)}
            for c in range(self.n_cores)
        ]


# ---------------------------------------------------------------------------
# Problem constants (hardcoded per the harness contract) and kernel builder.
# ---------------------------------------------------------------------------

D = 1024          # model dim
F = 2816          # expert hidden dim
E = 8             # experts == cores
TOPK = 2
DT = D // 128
FT = F // 128
BF16 = mybir.dt.bfloat16
FP32 = mybir.dt.float32
NPBF16 = mybir.dt.np(BF16)


def _tok_tiles(C):
    """Tiles of <=512 columns (PSUM bank width), all >=256 when possible
    (full-rate moving operands) and all even with even start offsets."""
    assert C % 2 == 0, C
    tiles, t0 = [], 0
    rem = C
    while rem > 768:
        tiles.append((t0, 512))
        t0 += 512
        rem -= 512
    if rem > 512:
        n1 = (rem - 256) & ~1
        tiles.append((t0, n1))
        tiles.append((t0 + n1, rem - n1))
    else:
        tiles.append((t0, rem))
    return tiles


def build(C=1072, n_copies=1):
    TOK = _tok_tiles(C)
    nc = bass.Bass(target_bir_lowering=False)
    xt = nc.dram_tensor("xt", [D, C], BF16, kind="ExternalInput")
    w12p = nc.dram_tensor("w12p", [FT, 128, 2 * DT * 128], BF16, kind="ExternalInput")
    w3p = nc.dram_tensor("w3p", [DT, 128, FT * 128], BF16, kind="ExternalInput")
    cw = nc.dram_tensor("cw", [128, C], FP32, kind="ExternalInput")
    yts = [nc.dram_tensor("yt" if i == 0 else f"yt{i}", [D, C], BF16,
                          kind="ExternalOutput") for i in range(n_copies)]

    with tile.TileContext(nc) as tc:
        with (
            tc.tile_pool(name="resident", bufs=1) as rpool,
            tc.tile_pool(name="stream", bufs=3) as spool,
            tc.tile_pool(name="work", bufs=2) as wpool,
            tc.tile_pool(name="psum", bufs=1, space="PSUM") as ppool,
        ):
            for yt in yts:
                xsb = rpool.tile([128, DT * C], BF16, tag="xsb")
                gsb = rpool.tile([128, FT * C], BF16, tag="gsb")
                cwsb = rpool.tile([128, C], FP32, tag="cwsb")

                # phase A: gT = silu((x @ w1.T).T) * (x @ w2.T).T
                for f in range(FT):
                    w12sb = spool.tile([128, 2 * DT * 128], BF16, tag="w12sb")
                    if f == 0:
                        # interleave the first weight tile with the x stream so
                        # the PE's first matmul can start ~2us in
                        nc.sync.dma_start(out=w12sb[:, 0:DT * 128],
                                          in_=w12p[0][:, 0:DT * 128])
                        nc.sync.dma_start(out=xsb[:, bass.ds(0, C)], in_=xt[0:128, :])
                        nc.sync.dma_start(out=w12sb[:, DT * 128:2 * DT * 128],
                                          in_=w12p[0][:, DT * 128:2 * DT * 128])
                        for k in range(1, DT):
                            nc.sync.dma_start(out=xsb[:, bass.ds(k * C, C)],
                                              in_=xt[k*128:(k+1)*128, :])
                        nc.sync.dma_start(out=cwsb[:, :], in_=cw[:, :])
                    else:
                        nc.sync.dma_start(out=w12sb[:, :], in_=w12p[f])
                    for (t0, tn) in TOK:
                        h1 = ppool.tile([128, 512], FP32, tag="h1", bufs=1)
                        h2 = ppool.tile([128, 512], FP32, tag="h2", bufs=1)
                        for k in range(DT):
                            nc.tensor.matmul(h1[:, :tn], w12sb[:, bass.ts(k, 128)],
                                             xsb[:, bass.ds(k * C + t0, tn)],
                                             start=(k == 0), stop=(k == DT - 1))
                        for k in range(DT):
                            nc.tensor.matmul(h2[:, :tn],
                                             w12sb[:, bass.ds(DT * 128 + k * 128, 128)],
                                             xsb[:, bass.ds(k * C + t0, tn)],
                                             start=(k == 0), stop=(k == DT - 1))
                        smu = wpool.tile([128, 512], FP32, tag="smu")
                        nc.scalar.activation(smu[:, :tn], h1[:, :tn],
                                             mybir.ActivationFunctionType.Silu)
                        nc.vector.tensor_mul(gsb[:, bass.ds(f * C + t0, tn)],
                                             smu[:, :tn], h2[:, :tn])

                # phase B: yT[d-block] = sum_f w3-block @ g-block, * combine
                for d in range(DT):
                    w3sb = spool.tile([128, FT * 128], BF16, tag="w3sb")
                    nc.sync.dma_start(out=w3sb[:, :], in_=w3p[d])
                    osb = wpool.tile([128, C], BF16, tag="osb", bufs=2)
                    for ti, (t0, tn) in enumerate(TOK):
                        yp = ppool.tile([128, 512], FP32, tag=f"yp{ti}", bufs=2)
                        for f in range(FT):
                            nc.tensor.matmul(yp[:, :tn], w3sb[:, bass.ts(f, 128)],
                                             gsb[:, bass.ds(f * C + t0, tn)],
                                             start=(f == 0), stop=(f == FT - 1))
                        nc.vector.tensor_mul(osb[:, bass.ds(t0, tn)],
                                             yp[:, :tn], cwsb[:, bass.ds(t0, tn)])
                        nc.sync.dma_start(out=yt[d*128:(d+1)*128, t0:t0 + tn],
                                          in_=osb[:, bass.ds(t0, tn)])
    return nc


# ---------------------------------------------------------------------------
# Host routing / dispatch / combine
# ---------------------------------------------------------------------------

def _route(x, gw):
    logits = x @ gw.T                                    # [T, E]
    order = np.argsort(-logits, axis=1, kind="stable")   # ties -> lower idx, as top_k
    idx = order[:, :TOPK]
    vals = np.take_along_axis(logits, idx, axis=1)
    ex = np.exp(vals - vals[:, :1])
    sv = ex / ex.sum(axis=1, keepdims=True)
    per_expert = []
    for e in range(E):
        mask = idx == e
        tok = np.nonzero(mask.any(axis=1))[0]
        per_expert.append((tok, sv[mask]))
    return per_expert


_runners = {}


def _get_runner(C):
    if C not in _runners:
        _runners[C] = SpmdRunner(build(C), E)
    return _runners[C]


def kernel(xmat, gw, w1, w2, w3):
    B, L, d = xmat.shape
    x = np.ascontiguousarray(np.asarray(xmat, dtype=np.float32).reshape(-1, d))
    gw = np.asarray(gw, dtype=np.float32)

    per_expert = _route(x, gw)
    max_n = max(len(tok) for tok, _ in per_expert)
    C = max(256, max_n + (max_n & 1))
    in_maps = []
    for e in range(E):
        tok, w = per_expert[e]
        n = len(tok)
        xtp = np.zeros((D, C), NPBF16)
        xtp[:, :n] = x[tok].T.astype(NPBF16)
        cwrow = np.zeros((1, C), np.float32)
        cwrow[0, :n] = w
        a1 = np.asarray(w1[e], np.float32).reshape(FT, 128, DT, 128).transpose(0, 3, 2, 1)
        a2 = np.asarray(w2[e], np.float32).reshape(FT, 128, DT, 128).transpose(0, 3, 2, 1)
        w12 = np.concatenate([a1.reshape(FT, 128, DT * 128),
                              a2.reshape(FT, 128, DT * 128)], axis=2).astype(NPBF16)
        b3 = np.asarray(w3[e], np.float32).reshape(DT, 128, FT, 128).transpose(0, 3, 2, 1)
        in_maps.append({
            "xt": xtp,
            "w12p": w12,
            "w3p": np.ascontiguousarray(b3.reshape(DT, 128, FT * 128)).astype(NPBF16),
            "cw": np.ascontiguousarray(np.broadcast_to(cwrow, (128, C))),
        })

    results = _get_runner(C).run(in_maps)

    y = np.zeros((x.shape[0], D), np.float32)
    for e in range(E):
        tok, _ = per_expert[e]
        y[tok] += results[e]["yt"][:, :len(tok)].T.astype(np.float32)
    return y.reshape(B, L, d)
